# revision 4
# baseline (speedup 1.0000x reference)
"""TRN2 Bass kernel for out = x @ W.T + b  (B=4096, IN=4096, OUT=4096, fp32).

Sharding (8 NeuronCores): 2-way along batch x 4-way along out_dim.
Per core: out_shard[2048, 1024] = x_shard[2048, 4096] @ W_shard[1024, 4096].T + b_shard.

Device kernel (per core, SPMD, no collectives):
  - W_shard^T kept resident in SBUF ([128, 32, 1024] fp32 = 16 MiB), loaded as
    4 DMAs (2 n-halves x 2 k-halves) so matmuls start before the full load.
  - x streamed in 16 chunks of 128 batch rows ([128, 32, 128], 2 MiB each,
    double buffered), host-side laid out so each chunk is one contiguous DMA.
  - PE: for each (m-chunk, n-tile of 512): 32 chained fp32r matmuls
    accumulating over K=4096 in one PSUM bank (full rate: 1 cycle/row).
  - Bias is broadcast once to [128, 1024] and added on PSUM->SBUF evict (DVE).

float32r: PE truncates each operand to 11 explicit mantissa bits (measured on
HW: exact for <=11-bit operands) but runs 4x faster than float32. Absmax error
vs fp32 reference ~1.5e-4 of output scale for K=4096.
"""

import numpy as np

import concourse.mybir as mybir
import concourse.tile as tile
from concourse import bacc
from concourse.bass_utils import run_bass_kernel_spmd

B = 4096
D_IN = 4096
D_OUT = 4096
R, C = 2, 4  # batch splits x out_dim splits (R*C = 8 cores)
BS = B // R  # 2048 batch rows per core
IS = D_OUT // C  # 1024 out cols per core
P = 128
KT = D_IN // P  # 32 k-tiles
MT = BS // P  # 16 m-chunks
NFREE = 512  # moving free dim per matmul (one fp32 PSUM bank)
NT = IS // NFREE  # 2 n-tiles
KH = 2  # k-halves for W load pipelining
F32R = mybir.dt.float32r
F32 = mybir.dt.float32


def build_nc(reps=1):
    nc = bacc.Bacc("TRN2", target_bir_lowering=False, debug=False)
    # Host pre-blocks inputs so every DMA source is contiguous:
    #   xb[mi, p, k, m] = x[mi*128 + m, k*128 + p]   (per-core batch rows)
    #   wb[p, k, i]     = W[i, k*128 + p]            (per-core out cols)
    xb = nc.dram_tensor("xb", [MT, P, KT, P], F32R, kind="ExternalInput")
    wb = nc.dram_tensor("wb", [P, KT, IS], F32R, kind="ExternalInput")
    bias = nc.dram_tensor("bias", [1, IS], F32, kind="ExternalInput")
    out = nc.dram_tensor("out", [BS, IS], F32, kind="ExternalOutput")

    with tile.TileContext(nc) as tc:
        with (
            tc.tile_pool(name="wpool", bufs=1) as wpool,
            tc.tile_pool(name="xpool", bufs=2) as xpool,
            tc.tile_pool(name="opool", bufs=4) as opool,
            tc.tile_pool(name="bpool", bufs=1) as bpool,
            tc.tile_pool(name="pspool", bufs=4, space="PSUM") as pspool,
        ):
            # bias broadcast to all 128 partitions, once
            brow = bpool.tile([1, IS], F32, tag="brow")
            bb = bpool.tile([P, IS], F32, tag="bb")
            nc.sync.dma_start(brow[:], bias[:])
            nc.gpsimd.partition_broadcast(bb[:], brow[:1, :])

            # resident W: w_tiles[ni][h] covers i in [ni*512, +512), k-half h
            w_tiles = []
            for ni in range(NT):
                row = []
                for h in range(KH):
                    wt = wpool.tile([P, KT // KH, NFREE], F32R, tag=f"w_{ni}_{h}")
                    nc.sync.dma_start(
                        wt[:],
                        wb[
                            :,
                            h * (KT // KH) : (h + 1) * (KT // KH),
                            ni * NFREE : (ni + 1) * NFREE,
                        ],
                    )
                    row.append(wt)
                w_tiles.append(row)

            def main_body(_iv=None):
                for mi in range(MT):
                    main_chunk(mi)

            def main_chunk(mi):
                xt = xpool.tile([P, KT, P], F32R, tag="x")
                nc.sync.dma_start(xt[:], xb[mi])
                for ni in range(NT):
                    ps = pspool.tile([P, NFREE], F32, tag="ps")
                    for k in range(KT):
                        h, kk = divmod(k, KT // KH)
                        nc.tensor.matmul(
                            ps[:],
                            xt[:, k, :],
                            w_tiles[ni][h][:, kk, :],
                            start=(k == 0),
                            stop=(k == KT - 1),
                        )
                    ot = opool.tile([P, NFREE], F32, tag="o")
                    nc.vector.tensor_add(
                        ot[:], ps[:], bb[:, ni * NFREE : (ni + 1) * NFREE]
                    )
                    nc.sync.dma_start(
                        out[mi * P : (mi + 1) * P, ni * NFREE : (ni + 1) * NFREE],
                        ot[:],
                    )

            if reps == 1:
                main_body()
            else:
                # benchmarking only: repeat the whole body in a HW loop
                with tc.For_i(0, reps, 1) as _i:
                    main_body(_i)
    nc.compile()
    return nc


_NC = None


def _get_nc():
    global _NC
    if _NC is None:
        _NC = build_nc()
    return _NC


def _shard_inputs(x, W, b):
    """Build per-core input maps (host-side layout prep, see build_nc)."""
    in_maps = []
    xb_by_r = []
    for r in range(R):
        xs = x[r * BS : (r + 1) * BS, :]  # [BS, D_IN]
        # [mi, m, k, p] -> [mi, p, k, m]
        blk = np.ascontiguousarray(
            xs.reshape(MT, P, KT, P).transpose(0, 3, 2, 1)
        ).astype(np.float32, copy=False)
        xb_by_r.append(blk)
    wb_by_c = []
    bias_by_c = []
    for c in range(C):
        ws = W[c * IS : (c + 1) * IS, :]  # [IS, D_IN]
        # [i, k, p] -> [p, k, i]
        blk = np.ascontiguousarray(ws.reshape(IS, KT, P).transpose(2, 1, 0)).astype(
            np.float32, copy=False
        )
        wb_by_c.append(blk)
        bias_by_c.append(
            np.ascontiguousarray(b[c * IS : (c + 1) * IS]).reshape(1, IS)
        )
    for core in range(R * C):
        r, c = divmod(core, C)
        in_maps.append(
            {"xb": xb_by_r[r], "wb": wb_by_c[c], "bias": bias_by_c[c]}
        )
    return in_maps


def run(x, W, b, trace=False, **spmd_kwargs):
    nc = _get_nc()
    in_maps = _shard_inputs(x, W, b)
    res = run_bass_kernel_spmd(
        nc, in_maps, core_ids=list(range(R * C)), trace=trace, **spmd_kwargs
    )
    out = np.empty((B, D_OUT), np.float32)
    for core in range(R * C):
        r, c = divmod(core, C)
        out[r * BS : (r + 1) * BS, c * IS : (c + 1) * IS] = res.results[core]["out"]
    return out, res


def kernel(x, W, b):
    x = np.asarray(x, dtype=np.float32)
    W = np.asarray(W, dtype=np.float32)
    b = np.asarray(b, dtype=np.float32)
    out, _ = run(x, W, b, trace=False)
    return out
